# revision 9
# baseline (speedup 1.0000x reference)
"""ColorHistogramLoss (soft histogram EMD) on 8 Trainium2 NeuronCores.

Strategy: pure data parallel over batch (B=8 -> one batch element per core).
Each core computes, for its 3 channels x {pred, target}, the 64-bin soft
(Gaussian-weighted) histogram of its 384x384 image.

Dense work = 64 Gaussian evals per pixel, split across ACT and DVE:

Layout: image as [32, 4608], replicated 4x across partition blocks of 32
rows -> [128, 4608]; block k = p//32 evaluates bins 16k + r for rounds
r = 0..15.  Work unit = half image [128, 2304], 12 units per core.

- ACT (scalar): per unit: tau = Exp((2s/D) x); seeds r=0,8 and dense rounds
  r=6,7,15 via Derivative_Erf(scale*x + bias[per-partition]) with fused
  accum_out free-dim reduction (~1.8us per pass).
- DVE (vector): the Gaussian ratio recurrence for the remaining 11 rounds:
      w_{j+1}(x) = w_j(x) * tau(x) * C_j,  C_j = exp(-s (c_j + c_{j+1})/D)
  as TWO bf16 passes per round:
      tensor_tensor:  m = w * tau            (2x_1P mode, 2 elem/cyc)
      tensor_scalar:  w' = m * C[partition]  + fused fp32 accum_out
                                             (4x_2P mode, 4 elem/cyc)
  ~1.1us per round vs 2.4us for the 1x scalar_tensor_tensor form.

Exp and Derivative_Erf live in different ACT table sets (~2.7us/switch), so
ACT batches per image pair: [Exp: 4x tau][DErf: 4x (s0,s8,d6,d7,d15)].

Per-partition partials land in hacc[128, 192]; two PE matmuls against the
4-block selector reduce over the 32 rows of each block; the tiny tail
(normalize, cumsum, |diff|, mean) runs on host in float64.
"""

import functools
import math

import numpy as np

N_CORES = 8
NUM_BINS = 64
B, C, H, W = 8, 3, 384, 384
HW = H * W
N_UNITS = 2 * C                 # (channel, pred/target) images per core
NREP = 4                        # replication factor
ROWS = 128 // NREP              # 32 image rows per replica block
IMG_FREE = HW // ROWS           # 4608
NQ = 2                          # halves per image
QF = IMG_FREE // NQ             # 2304
UNITS = N_UNITS * NQ            # 12 work units per core per iteration
NROUNDS = NUM_BINS // NREP      # 16 rounds per unit

DENOM = 2.0 * (1.0 / 64.0) ** 2 + 1e-7
SCALE = 1.0 / math.sqrt(DENOM)          # Derivative_Erf arg scale
SPACING = 1.0 / 63.0                    # bin-center spacing
TAU_SCALE = 2.0 * SPACING / DENOM       # tau = exp(TAU_SCALE * x)
DERF_SCALE = math.sqrt(math.pi) / 2.0   # Derivative_Erf = 2/sqrt(pi)*exp(-u^2)

SEED_ROUNDS = (0, 8)
DENSE_ROUNDS = (5, 6, 7, 13, 14, 15)
CHAIN_ROUNDS = (1, 2, 3, 4, 9, 10, 11, 12)

HACC_COLS = UNITS * NROUNDS     # 192

# consts column layout
CBIAS = {r: i for i, r in enumerate(SEED_ROUNDS + DENSE_ROUNDS)}  # DErf biases
CC = {r: 7 + r for r in range(1, 13)}       # C_{j-1->j} cols 8..19
CSEL = 20                                   # selector cols 20..23
NCONST = 24

ACT_KINDS = ("s0", "s8") + tuple(f"d{r}" for r in DENSE_ROUNDS)


def _act_rounds():
    return ("tau",) + ACT_KINDS


def _dve_ops():
    ops = []
    for r in CHAIN_ROUNDS:
        ops += [(f"tt{r}",), (f"ts{r}",)]
    return tuple(o[0] for o in ops)


DVE_OPS = _dve_ops()            # 22 ops per unit
ACT_LAST = "d15"
DVE_LAST = DVE_OPS[-1]          # ts14


def _build_schedules(R):
    act_prog, dve_prog = [], []
    for it in range(R):
        base = UNITS * it
        for pair in range(UNITS // 4):      # 4 units = 2 images per pair
            gs = [base + 4 * pair + j for j in range(4)]
            for g in gs:
                act_prog.append(("tau", g))
            for g in gs:
                act_prog += [(k, g) for k in ACT_KINDS]
        for u in range(UNITS):
            g = base + u
            dve_prog += [(k, g) for k in DVE_OPS]
    act_index = {key: idx for idx, key in enumerate(act_prog)}
    dve_index = {key: idx for idx, key in enumerate(dve_prog)}
    return act_prog, dve_prog, act_index, dve_index


def _build_program(R=1):
    import concourse.bass as bass
    import concourse.mybir as mybir
    from contextlib import ExitStack

    act_prog, dve_prog, act_index, dve_index = _build_schedules(R)
    act_per_it = len(act_prog) // R
    dve_per_it = len(dve_prog) // R

    nc = bass.Bass()
    xs = [
        nc.dram_tensor(
            f"x{u}", [128, IMG_FREE], mybir.dt.float32, kind="ExternalInput"
        )
        for u in range(N_UNITS)
    ]
    cst = nc.dram_tensor(
        "consts", [128, NCONST], mybir.dt.float32, kind="ExternalInput"
    )
    hist_out = nc.dram_tensor(
        "hist", [128, 8], mybir.dt.float32, kind="ExternalOutput"
    )

    mult = mybir.AluOpType.mult
    bf16 = mybir.dt.bfloat16

    with ExitStack() as stack:
        def sb(name, shape, dt=mybir.dt.float32):
            return stack.enter_context(nc.sbuf_tensor(name, shape, dt))

        xts = [sb(f"xt{i}", [128, QF]) for i in range(8)]
        taus = [sb(f"tau{i}", [128, QF], bf16) for i in range(8)]
        sas = [sb(f"sa{i}", [128, QF], bf16) for i in range(2)]
        sbs = [sb(f"sb{i}", [128, QF], bf16) for i in range(2)]
        pps = [sb(f"pp{i}", [128, QF], bf16) for i in range(2)]
        mms = [sb(f"mm{i}", [128, QF], bf16) for i in range(2)]
        wscr = sb("wscr", [128, QF], bf16)
        cstt = sb("cstt", [128, NCONST])
        hacc = sb("hacc", [128, HACC_COLS])
        ho = sb("ho", [128, 8])
        ph0 = stack.enter_context(nc.psum_tensor("ph0", [128, 4], mybir.dt.float32))
        ph1 = stack.enter_context(nc.psum_tensor("ph1", [64, 4], mybir.dt.float32))
        sem_c = stack.enter_context(nc.semaphore("sem_c"))
        xsems = [stack.enter_context(nc.semaphore(f"sem_x{i}")) for i in range(8)]
        act_sem = stack.enter_context(nc.semaphore("act_sem"))
        dve_sem = stack.enter_context(nc.semaphore("dve_sem"))
        pe_sem = stack.enter_context(nc.semaphore("pe_sem"))
        cp_sem = stack.enter_context(nc.semaphore("cp_sem"))
        block = stack.enter_context(nc.Block())

        def haccol(g, rnd):
            c = NROUNDS * (g % UNITS) + rnd
            return hacc[:, c : c + 1]

        @block.sync
        def _(sync):
            sync.dma_start(out=cstt[:], in_=cst[:]).then_inc(sem_c, 16)
            for it in range(R):
                for u in range(UNITS):
                    g = UNITS * it + u
                    slot = g % 8
                    i, q = (g % UNITS) // NQ, g % NQ
                    if g >= 8:
                        sync.wait_ge(act_sem, act_index[(ACT_LAST, g - 8)] + 1)
                    sync.dma_start(
                        out=xts[slot][:], in_=xs[i][:, QF * q : QF * (q + 1)]
                    ).then_inc(xsems[slot], 16)
                sync.wait_ge(cp_sem, 2 * (it + 1))
                sync.dma_start(out=hist_out[:], in_=ho[:]).then_inc(sem_c, 16)

        @block.scalar
        def _(scalar):
            # dummy activation: pulls the exp table load forward
            scalar.activation(
                wscr[0:128, 0:1], wscr[0:128, 1:2],
                mybir.ActivationFunctionType.Exp,
                bias=0.0, scale=1.0,
            )
            scalar.wait_ge(sem_c, 16)
            for kind, g in act_prog:
                slot, b = g % 8, g % 2
                it = g // UNITS
                scalar.wait_ge(xsems[slot], 16 * (g // 8 + 1))
                if kind == "tau":
                    if g >= 8:
                        scalar.wait_ge(dve_sem, dve_index[(DVE_LAST, g - 8)] + 1)
                    ins = scalar.activation(
                        taus[slot][:], xts[slot][:],
                        mybir.ActivationFunctionType.Exp,
                        bias=0.0, scale=float(TAU_SCALE),
                    )
                elif kind in ("s0", "s8"):
                    rnd = int(kind[1:])
                    if kind == "s0" and g % UNITS == 0 and it > 0:
                        scalar.wait_ge(pe_sem, it)
                    if g >= 2:
                        scalar.wait_ge(dve_sem, dve_index[(DVE_LAST, g - 2)] + 1)
                    dst = sas[b] if kind == "s0" else sbs[b]
                    cb = CBIAS[rnd]
                    ins = scalar.activation(
                        dst[:], xts[slot][:],
                        mybir.ActivationFunctionType.Derivative_Erf,
                        bias=cstt[:, cb : cb + 1], scale=float(SCALE),
                        accum_out=haccol(g, rnd),
                    )
                else:  # dense d6/d7/d15
                    rnd = int(kind[1:])
                    cb = CBIAS[rnd]
                    ins = scalar.activation(
                        wscr[:], xts[slot][:],
                        mybir.ActivationFunctionType.Derivative_Erf,
                        bias=cstt[:, cb : cb + 1], scale=float(SCALE),
                        accum_out=haccol(g, rnd),
                    )
                ins.then_inc(act_sem, 1)

        @block.vector
        def _(vector):
            for kind, g in dve_prog:
                slot, b = g % 8, g % 2
                rnd = int(kind[2:])
                chain = "A" if rnd < 8 else "B"
                step = rnd - (0 if chain == "A" else 8)   # 1..5 / 1..6
                base = sas[b] if chain == "A" else sbs[b]
                src = base if step % 2 == 1 else pps[b]
                dst = pps[b] if step % 2 == 1 else base
                if kind.startswith("tt"):
                    if rnd == 1:
                        vector.wait_ge(act_sem, act_index[("s0", g)] + 1)
                    elif rnd == 9:
                        vector.wait_ge(act_sem, act_index[("s8", g)] + 1)
                    ins = vector.tensor_tensor(
                        mms[b][:], src[:], taus[slot][:], mult
                    )
                else:  # ts: w' = (m * C_r) + 0  (+ fused accum)
                    cc = CC[rnd]
                    ins = vector.tensor_scalar(
                        dst[:], mms[b][:], cstt[:, cc : cc + 1], 0.0, mult,
                        mybir.AluOpType.add,
                        accum_out=haccol(g, rnd),
                    )
                ins.then_inc(dve_sem, 1)
                if kind == DVE_LAST and g % UNITS == UNITS - 1:
                    it = g // UNITS
                    vector.wait_ge(pe_sem, it + 1)
                    vector.wait_ge(sem_c, 16 * (it + 1))
                    vector.tensor_copy(ho[:, 0:4], ph0[:, :]).then_inc(cp_sem, 1)
                    vector.tensor_copy(ho[0:64, 4:8], ph1[:, :]).then_inc(
                        cp_sem, 1
                    )

        @block.tensor
        def _(tensor):
            for it in range(R):
                tensor.wait_ge(act_sem, act_per_it * (it + 1))
                tensor.wait_ge(dve_sem, dve_per_it * (it + 1))
                tensor.matmul(
                    ph0[0:128, 0:4], hacc[:, 0:128], cstt[:, CSEL : CSEL + 4],
                    start=True, stop=True,
                )
                tensor.matmul(
                    ph1[0:64, 0:4], hacc[:, 128:192], cstt[:, CSEL : CSEL + 4],
                    start=True, stop=True,
                ).then_inc(pe_sem, 1)

    return nc


def _make_consts():
    centers = np.linspace(0.0, 1.0, NUM_BINS).astype(np.float64)
    p = np.arange(128)
    k = p // ROWS
    cst = np.zeros((128, NCONST), dtype=np.float64)
    for r, ci in CBIAS.items():
        cst[:, ci] = -centers[NROUNDS * k + r] * SCALE
    for r, ci in CC.items():
        j = NROUNDS * k + r
        cst[:, ci] = np.exp(-SPACING * (centers[j - 1] + centers[j]) / DENOM)
    for kk in range(NREP):
        cst[k == kk, CSEL + kk] = 1.0
    return cst.astype(np.float32)


@functools.lru_cache(maxsize=1)
def _get_runner():
    """Compile the SPMD program once; return a callable list[in_map] -> list[out_map]."""
    import jax
    from jax.experimental.shard_map import shard_map
    from jax.sharding import Mesh, PartitionSpec

    from concourse import mybir
    from concourse.bass2jax import (
        _bass_exec_p,
        install_neuronx_cc_hook,
        partition_id_tensor,
    )

    nc = _build_program()
    install_neuronx_cc_hook()

    partition_name = (
        nc.partition_id_tensor.name if nc.partition_id_tensor else None
    )
    in_names, out_names, out_avals, zero_outs = [], [], [], []
    for alloc in nc.m.functions[0].allocations:
        if not isinstance(alloc, mybir.MemoryLocationSet):
            continue
        name = alloc.memorylocations[0].name
        if alloc.kind == "ExternalInput":
            if name != partition_name:
                in_names.append(name)
        elif alloc.kind == "ExternalOutput":
            out_names.append(name)
            shape = tuple(alloc.tensor_shape)
            dtype = mybir.dt.np(alloc.dtype)
            out_avals.append(jax.core.ShapedArray(shape, dtype))
            zero_outs.append(np.zeros(shape, dtype))
    n_params = len(in_names)
    n_outs = len(out_avals)
    all_in_names = list(in_names) + list(out_names)
    if partition_name is not None:
        all_in_names.append(partition_name)
    donate = tuple(range(n_params, n_params + n_outs))

    def _body(*args):
        operands = list(args)
        if partition_name is not None:
            operands.append(partition_id_tensor())
        outs = _bass_exec_p.bind(
            *operands,
            out_avals=tuple(out_avals),
            in_names=tuple(all_in_names),
            out_names=tuple(out_names),
            lowering_input_output_aliases=(),
            sim_require_finite=True,
            sim_require_nnan=True,
            nc=nc,
        )
        return tuple(outs)

    devices = jax.devices()[:N_CORES]
    mesh = Mesh(np.asarray(devices), ("core",))
    sharded = jax.jit(
        shard_map(
            _body,
            mesh=mesh,
            in_specs=(PartitionSpec("core"),) * (n_params + n_outs),
            out_specs=(PartitionSpec("core"),) * n_outs,
            check_rep=False,
        ),
        donate_argnums=donate,
        keep_unused=True,
    )

    class Runner:
        def __init__(self):
            self.sharded = sharded
            self.in_names = in_names
            self.out_names = out_names
            self.out_avals = out_avals
            self.zero_outs = zero_outs

        def concat_inputs(self, in_maps):
            return [
                np.concatenate([np.asarray(m[name]) for m in in_maps], axis=0)
                for name in in_names
            ]

        def fresh_zeros(self):
            return [
                np.zeros((N_CORES * z.shape[0], *z.shape[1:]), z.dtype)
                for z in zero_outs
            ]

        def split_outputs(self, out_arrs):
            return [
                {
                    name: np.asarray(out_arrs[i]).reshape(
                        N_CORES, *out_avals[i].shape
                    )[c]
                    for i, name in enumerate(out_names)
                }
                for c in range(N_CORES)
            ]

        def __call__(self, in_maps):
            out_arrs = self.sharded(*self.concat_inputs(in_maps), *self.fresh_zeros())
            return self.split_outputs(out_arrs)

    return Runner()


def _shard_inputs(pred, target):
    cst = _make_consts()
    maps = []
    for b in range(B):
        m = {"consts": cst}
        for c in range(C):
            for t, src in enumerate((pred, target)):
                u = 2 * c + t
                img = np.ascontiguousarray(src[b, c], dtype=np.float32).reshape(
                    ROWS, IMG_FREE
                )
                m[f"x{u}"] = np.tile(img, (NREP, 1))
        maps.append(m)
    return maps


def _unpack_hist(ho):
    """ho [128, 8] -> hist [N_UNITS, NUM_BINS] (float64)."""
    ho = ho.astype(np.float64)
    hist = np.zeros((N_UNITS, NUM_BINS), dtype=np.float64)
    for u in range(UNITS):
        i = u // NQ
        for rnd in range(NROUNDS):
            c = NROUNDS * u + rnd
            vals = ho[c, 0:4] if c < 128 else ho[c - 128, 4:8]
            for kk in range(NREP):
                hist[i, NROUNDS * kk + rnd] += vals[kk]
    return hist


def _finish_on_host(results):
    total = 0.0
    for b in range(B):
        hist = _unpack_hist(results[b]["hist"]) * DERF_SCALE
        for c in range(C):
            pcs = hist[2 * c]
            tcs = hist[2 * c + 1]
            pn = pcs / (pcs.sum() + 1e-7)
            tn = tcs / (tcs.sum() + 1e-7)
            total += np.abs(np.cumsum(pn) - np.cumsum(tn)).sum()
    return np.float32(total / (B * C * NUM_BINS))


def kernel(pred, target):
    pred = np.asarray(pred, dtype=np.float32)
    target = np.asarray(target, dtype=np.float32)
    assert pred.shape == (B, C, H, W) and target.shape == (B, C, H, W)
    run = _get_runner()
    results = run(_shard_inputs(pred, target))
    return np.asarray(_finish_on_host(results), dtype=np.float32)


# revision 13
# speedup vs baseline: 1.3986x; 1.3986x over previous
"""ColorHistogramLoss (soft histogram EMD) on 8 Trainium2 NeuronCores.

Strategy: pure data parallel over batch (B=8 -> one batch element per core).
Each core computes, for its 3 channels x {pred, target}, the 64-bin soft
(Gaussian-weighted) histogram of its 384x384 image.

Dense work = 64 Gaussian evals per pixel, split across ACT and DVE:

Layout: image as [32, 4608], replicated 4x across partition blocks of 32
rows -> [128, 4608]; block k = p//32 evaluates bins 16k + r for rounds
r = 0..15.  Work unit = half image [128, 2304], 12 units per core.

- ACT (scalar): per unit: tau = Exp((2s/D) x); seeds r=0,8 and dense rounds
  r=6,7,15 via Derivative_Erf(scale*x + bias[per-partition]) with fused
  accum_out free-dim reduction (~1.8us per pass).
- DVE (vector): the Gaussian ratio recurrence for the remaining 11 rounds:
      w_{j+1}(x) = w_j(x) * tau(x) * C_j,  C_j = exp(-s (c_j + c_{j+1})/D)
  as TWO bf16 passes per round:
      tensor_tensor:  m = w * tau            (2x_1P mode, 2 elem/cyc)
      tensor_scalar:  w' = m * C[partition]  + fused fp32 accum_out
                                             (4x_2P mode, 4 elem/cyc)
  ~1.1us per round vs 2.4us for the 1x scalar_tensor_tensor form.

Exp and Derivative_Erf live in different ACT table sets (~2.7us/switch), so
ACT batches per image pair: [Exp: 4x tau][DErf: 4x (s0,s8,d6,d7,d15)].

Per-partition partials land in hacc[128, 192]; two PE matmuls against the
4-block selector reduce over the 32 rows of each block; the tiny tail
(normalize, cumsum, |diff|, mean) runs on host in float64.
"""

import functools
import math

import numpy as np

N_CORES = 8
NUM_BINS = 64
B, C, H, W = 8, 3, 384, 384
HW = H * W
N_UNITS = 2 * C                 # (channel, pred/target) images per core
NREP = 4                        # replication factor
ROWS = 128 // NREP              # 32 image rows per replica block
IMG_FREE = HW // ROWS           # 4608
NQ = 2                          # halves per image
QF = IMG_FREE // NQ             # 2304
UNITS = N_UNITS * NQ            # 12 work units per core per iteration
NROUNDS = NUM_BINS // NREP      # 16 rounds per unit

DENOM = 2.0 * (1.0 / 64.0) ** 2 + 1e-7
SCALE = 1.0 / math.sqrt(DENOM)          # Derivative_Erf arg scale
SPACING = 1.0 / 63.0                    # bin-center spacing
TAU_SCALE = 2.0 * SPACING / DENOM       # tau = exp(TAU_SCALE * x)
DERF_SCALE = math.sqrt(math.pi) / 2.0   # Derivative_Erf = 2/sqrt(pi)*exp(-u^2)

SEED_ROUNDS = (0, 8)
DENSE_ROUNDS = (4, 5, 6, 7, 13, 14, 15)
CHAIN_ROUNDS = (1, 2, 3, 9, 10, 11, 12)

HACC_COLS = UNITS * NROUNDS     # 192

# consts column layout
CBIAS = {r: i for i, r in enumerate(SEED_ROUNDS + DENSE_ROUNDS)}  # DErf biases
CC = {r: 8 + r for r in range(1, 13)}       # C_{j-1->j} cols 9..20
CSEL = 21                                   # selector cols 21..24
NCONST = 28

ACT_KINDS = ("s0", "s8") + tuple(f"d{r}" for r in DENSE_ROUNDS)


def _act_rounds():
    return ("tau",) + ACT_KINDS


DVE_OPS = tuple(f"st{r}" for r in CHAIN_ROUNDS)     # 7 ops per unit
ACT_LAST = "d15"
DVE_LAST = DVE_OPS[-1]          # st12
GROUP = 6                       # units per ACT table-set batch (3 images)


def _build_schedules(R):
    act_prog, dve_prog = [], []
    for it in range(R):
        base = UNITS * it
        for grp in range(UNITS // GROUP):
            gs = [base + GROUP * grp + j for j in range(GROUP)]
            for g in gs:
                act_prog.append(("tau", g))
            for g in gs:
                act_prog += [(k, g) for k in ACT_KINDS]
        for u in range(UNITS):
            g = base + u
            dve_prog += [(k, g) for k in DVE_OPS]
    act_index = {key: idx for idx, key in enumerate(act_prog)}
    dve_index = {key: idx for idx, key in enumerate(dve_prog)}
    return act_prog, dve_prog, act_index, dve_index


def _build_program(R=1):
    import concourse.bass as bass
    import concourse.mybir as mybir
    from contextlib import ExitStack

    act_prog, dve_prog, act_index, dve_index = _build_schedules(R)
    act_per_it = len(act_prog) // R
    dve_per_it = len(dve_prog) // R

    nc = bass.Bass()
    xs = [
        nc.dram_tensor(
            f"x{u}", [128, IMG_FREE], mybir.dt.float32, kind="ExternalInput"
        )
        for u in range(N_UNITS)
    ]
    cst = nc.dram_tensor(
        "consts", [128, NCONST], mybir.dt.float32, kind="ExternalInput"
    )
    hist_out = nc.dram_tensor(
        "hist", [128, 8], mybir.dt.float32, kind="ExternalOutput"
    )

    mult = mybir.AluOpType.mult
    bf16 = mybir.dt.bfloat16

    with ExitStack() as stack:
        def sb(name, shape, dt=mybir.dt.float32):
            return stack.enter_context(nc.sbuf_tensor(name, shape, dt))

        xts = [sb(f"xt{i}", [128, QF]) for i in range(8)]
        taus = [sb(f"tau{i}", [128, QF], bf16) for i in range(12)]
        sas = [sb(f"sa{i}", [128, QF], bf16) for i in range(2)]
        sbs = [sb(f"sb{i}", [128, QF], bf16) for i in range(2)]
        pps = [sb(f"pp{i}", [128, QF], bf16) for i in range(2)]
        wscr = sb("wscr", [128, QF], bf16)
        cstt = sb("cstt", [128, NCONST])
        hacc = sb("hacc", [128, HACC_COLS])
        ho = sb("ho", [128, 8])
        ph0 = stack.enter_context(nc.psum_tensor("ph0", [128, 4], mybir.dt.float32))
        ph1 = stack.enter_context(nc.psum_tensor("ph1", [64, 4], mybir.dt.float32))
        sem_c = stack.enter_context(nc.semaphore("sem_c"))
        xsems = [stack.enter_context(nc.semaphore(f"sem_x{i}")) for i in range(8)]
        act_sem = stack.enter_context(nc.semaphore("act_sem"))
        dve_sem = stack.enter_context(nc.semaphore("dve_sem"))
        pe_sem = stack.enter_context(nc.semaphore("pe_sem"))
        cp_sem = stack.enter_context(nc.semaphore("cp_sem"))
        block = stack.enter_context(nc.Block())

        def haccol(g, rnd):
            c = NROUNDS * (g % UNITS) + rnd
            return hacc[:, c : c + 1]

        @block.sync
        def _(sync):
            sync.dma_start(out=cstt[:], in_=cst[:]).then_inc(sem_c, 16)
            for it in range(R):
                for u in range(UNITS):
                    g = UNITS * it + u
                    slot = g % 8
                    i, q = (g % UNITS) // NQ, g % NQ
                    if g >= 8:
                        sync.wait_ge(act_sem, act_index[(ACT_LAST, g - 8)] + 1)
                    sync.dma_start(
                        out=xts[slot][:], in_=xs[i][:, QF * q : QF * (q + 1)]
                    ).then_inc(xsems[slot], 16)
                sync.wait_ge(cp_sem, 2 * (it + 1))
                sync.dma_start(out=hist_out[:], in_=ho[:]).then_inc(sem_c, 16)

        @block.scalar
        def _(scalar):
            # dummy activation: pulls the exp table load forward
            scalar.activation(
                wscr[0:128, 0:1], wscr[0:128, 1:2],
                mybir.ActivationFunctionType.Exp,
                bias=0.0, scale=1.0,
            )
            scalar.wait_ge(sem_c, 16)
            for kind, g in act_prog:
                slot, b = g % 8, g % 2
                tslot = g % 12
                it = g // UNITS
                scalar.wait_ge(xsems[slot], 16 * (g // 8 + 1))
                if kind == "tau":
                    if g >= 12:
                        scalar.wait_ge(dve_sem, dve_index[(DVE_LAST, g - 12)] + 1)
                    ins = scalar.activation(
                        taus[tslot][:], xts[slot][:],
                        mybir.ActivationFunctionType.Exp,
                        bias=0.0, scale=float(TAU_SCALE),
                    )
                elif kind in ("s0", "s8"):
                    rnd = int(kind[1:])
                    if kind == "s0" and g % UNITS == 0 and it > 0:
                        scalar.wait_ge(pe_sem, it)
                    if g >= 2:
                        scalar.wait_ge(dve_sem, dve_index[(DVE_LAST, g - 2)] + 1)
                    dst = sas[b] if kind == "s0" else sbs[b]
                    cb = CBIAS[rnd]
                    ins = scalar.activation(
                        dst[:], xts[slot][:],
                        mybir.ActivationFunctionType.Derivative_Erf,
                        bias=cstt[:, cb : cb + 1], scale=float(SCALE),
                        accum_out=haccol(g, rnd),
                    )
                else:  # dense d6/d7/d15
                    rnd = int(kind[1:])
                    cb = CBIAS[rnd]
                    ins = scalar.activation(
                        wscr[:], xts[slot][:],
                        mybir.ActivationFunctionType.Derivative_Erf,
                        bias=cstt[:, cb : cb + 1], scale=float(SCALE),
                        accum_out=haccol(g, rnd),
                    )
                ins.then_inc(act_sem, 1)

        @block.vector
        def _(vector):
            for kind, g in dve_prog:
                tslot, b = g % 12, g % 2
                rnd = int(kind[2:])
                chain = "A" if rnd < 8 else "B"
                step = rnd - (0 if chain == "A" else 8)   # 1..3 / 1..4
                base = sas[b] if chain == "A" else sbs[b]
                src = base if step % 2 == 1 else pps[b]
                dst = pps[b] if step % 2 == 1 else base
                if rnd == 1:
                    vector.wait_ge(act_sem, act_index[("s0", g)] + 1)
                elif rnd == 9:
                    vector.wait_ge(act_sem, act_index[("s8", g)] + 1)
                cc = CC[rnd]
                # w' = (w * C_r) * tau  with fused fp32 accum
                ins = vector.scalar_tensor_tensor(
                    dst[:], src[:], cstt[:, cc : cc + 1], taus[tslot][:],
                    mult, mult,
                    accum_out=haccol(g, rnd),
                )
                ins.then_inc(dve_sem, 1)
                if kind == DVE_LAST and g % UNITS == UNITS - 1:
                    it = g // UNITS
                    vector.wait_ge(pe_sem, it + 1)
                    vector.wait_ge(sem_c, 16 * (it + 1))
                    vector.tensor_copy(ho[:, 0:4], ph0[:, :]).then_inc(cp_sem, 1)
                    vector.tensor_copy(ho[0:64, 4:8], ph1[:, :]).then_inc(
                        cp_sem, 1
                    )

        @block.tensor
        def _(tensor):
            for it in range(R):
                tensor.wait_ge(act_sem, act_per_it * (it + 1))
                tensor.wait_ge(dve_sem, dve_per_it * (it + 1))
                tensor.matmul(
                    ph0[0:128, 0:4], hacc[:, 0:128], cstt[:, CSEL : CSEL + 4],
                    start=True, stop=True,
                )
                tensor.matmul(
                    ph1[0:64, 0:4], hacc[:, 128:192], cstt[:, CSEL : CSEL + 4],
                    start=True, stop=True,
                ).then_inc(pe_sem, 1)

    return nc


def _make_consts():
    centers = np.linspace(0.0, 1.0, NUM_BINS).astype(np.float64)
    p = np.arange(128)
    k = p // ROWS
    cst = np.zeros((128, NCONST), dtype=np.float64)
    for r, ci in CBIAS.items():
        cst[:, ci] = -centers[NROUNDS * k + r] * SCALE
    for r, ci in CC.items():
        j = NROUNDS * k + r
        cst[:, ci] = np.exp(-SPACING * (centers[j - 1] + centers[j]) / DENOM)
    for kk in range(NREP):
        cst[k == kk, CSEL + kk] = 1.0
    return cst.astype(np.float32)


@functools.lru_cache(maxsize=1)
def _get_runner():
    """Compile the SPMD program once; return a callable list[in_map] -> list[out_map]."""
    import jax
    from jax.experimental.shard_map import shard_map
    from jax.sharding import Mesh, PartitionSpec

    from concourse import mybir
    from concourse.bass2jax import (
        _bass_exec_p,
        install_neuronx_cc_hook,
        partition_id_tensor,
    )

    nc = _build_program()
    install_neuronx_cc_hook()

    partition_name = (
        nc.partition_id_tensor.name if nc.partition_id_tensor else None
    )
    in_names, out_names, out_avals, zero_outs = [], [], [], []
    for alloc in nc.m.functions[0].allocations:
        if not isinstance(alloc, mybir.MemoryLocationSet):
            continue
        name = alloc.memorylocations[0].name
        if alloc.kind == "ExternalInput":
            if name != partition_name:
                in_names.append(name)
        elif alloc.kind == "ExternalOutput":
            out_names.append(name)
            shape = tuple(alloc.tensor_shape)
            dtype = mybir.dt.np(alloc.dtype)
            out_avals.append(jax.core.ShapedArray(shape, dtype))
            zero_outs.append(np.zeros(shape, dtype))
    n_params = len(in_names)
    n_outs = len(out_avals)
    all_in_names = list(in_names) + list(out_names)
    if partition_name is not None:
        all_in_names.append(partition_name)
    donate = tuple(range(n_params, n_params + n_outs))

    def _body(*args):
        operands = list(args)
        if partition_name is not None:
            operands.append(partition_id_tensor())
        outs = _bass_exec_p.bind(
            *operands,
            out_avals=tuple(out_avals),
            in_names=tuple(all_in_names),
            out_names=tuple(out_names),
            lowering_input_output_aliases=(),
            sim_require_finite=True,
            sim_require_nnan=True,
            nc=nc,
        )
        return tuple(outs)

    devices = jax.devices()[:N_CORES]
    mesh = Mesh(np.asarray(devices), ("core",))
    sharded = jax.jit(
        shard_map(
            _body,
            mesh=mesh,
            in_specs=(PartitionSpec("core"),) * (n_params + n_outs),
            out_specs=(PartitionSpec("core"),) * n_outs,
            check_rep=False,
        ),
        donate_argnums=donate,
        keep_unused=True,
    )

    class Runner:
        def __init__(self):
            self.sharded = sharded
            self.in_names = in_names
            self.out_names = out_names
            self.out_avals = out_avals
            self.zero_outs = zero_outs

        def concat_inputs(self, in_maps):
            return [
                np.concatenate([np.asarray(m[name]) for m in in_maps], axis=0)
                for name in in_names
            ]

        def fresh_zeros(self):
            return [
                np.zeros((N_CORES * z.shape[0], *z.shape[1:]), z.dtype)
                for z in zero_outs
            ]

        def split_outputs(self, out_arrs):
            return [
                {
                    name: np.asarray(out_arrs[i]).reshape(
                        N_CORES, *out_avals[i].shape
                    )[c]
                    for i, name in enumerate(out_names)
                }
                for c in range(N_CORES)
            ]

        def __call__(self, in_maps):
            out_arrs = self.sharded(*self.concat_inputs(in_maps), *self.fresh_zeros())
            return self.split_outputs(out_arrs)

    return Runner()


def _shard_inputs(pred, target):
    cst = _make_consts()
    maps = []
    for b in range(B):
        m = {"consts": cst}
        for c in range(C):
            for t, src in enumerate((pred, target)):
                u = 2 * c + t
                img = np.ascontiguousarray(src[b, c], dtype=np.float32).reshape(
                    ROWS, IMG_FREE
                )
                m[f"x{u}"] = np.tile(img, (NREP, 1))
        maps.append(m)
    return maps


def _unpack_hist(ho):
    """ho [128, 8] -> hist [N_UNITS, NUM_BINS] (float64)."""
    ho = ho.astype(np.float64)
    hist = np.zeros((N_UNITS, NUM_BINS), dtype=np.float64)
    for u in range(UNITS):
        i = u // NQ
        for rnd in range(NROUNDS):
            c = NROUNDS * u + rnd
            vals = ho[c, 0:4] if c < 128 else ho[c - 128, 4:8]
            for kk in range(NREP):
                hist[i, NROUNDS * kk + rnd] += vals[kk]
    return hist


def _finish_on_host(results):
    total = 0.0
    for b in range(B):
        hist = _unpack_hist(results[b]["hist"]) * DERF_SCALE
        for c in range(C):
            pcs = hist[2 * c]
            tcs = hist[2 * c + 1]
            pn = pcs / (pcs.sum() + 1e-7)
            tn = tcs / (tcs.sum() + 1e-7)
            total += np.abs(np.cumsum(pn) - np.cumsum(tn)).sum()
    return np.float32(total / (B * C * NUM_BINS))


def kernel(pred, target):
    pred = np.asarray(pred, dtype=np.float32)
    target = np.asarray(target, dtype=np.float32)
    assert pred.shape == (B, C, H, W) and target.shape == (B, C, H, W)
    run = _get_runner()
    results = run(_shard_inputs(pred, target))
    return np.asarray(_finish_on_host(results), dtype=np.float32)


# revision 17
# speedup vs baseline: 1.6140x; 1.1539x over previous
"""ColorHistogramLoss (soft histogram EMD) on 8 Trainium2 NeuronCores.

Strategy: pure data parallel over batch (B=8 -> one batch element per core).
Each core computes, for its 3 channels x {pred, target}, the 64-bin soft
(Gaussian-weighted) histogram of its 384x384 image.

Dense work = 64 Gaussian evals per pixel, split across ACT and DVE:

Layout: image as [32, 4608], replicated 4x across partition blocks of 32
rows -> [128, 4608]; block k = p//32 evaluates bins 16k + r for rounds
r = 0..15.  Work unit = half image [128, 2304], 12 units per core.

- ACT (scalar): per unit: tau = Exp((2s/D) x); seeds r=0,8 and 6-7 dense
  rounds via Derivative_Erf(scale*x + bias[per-partition]) with fused
  accum_out free-dim reduction (~1.8us per pass, measured).
- DVE (vector): the Gaussian ratio recurrence for the remaining 7-8 rounds:
      w_{j+1}(x) = w_j(x) * tau(x) * C_j,  C_j = exp(-s (c_j + c_{j+1})/D)
  as ONE fused bf16 scalar_tensor_tensor pass per round:
      w' = (w * C[per-partition]) * tau,  accum_out = fp32 sum
  (~2.4us per round; STT always runs at 1x -- measured.  A TT-2x + TS-4x
  two-instruction split was tried and measured SLOWER (~3.4us/round): the
  TT's 2x_1P mode does not engage on HW / alternating DVE opcodes stall,
  so do not reintroduce it.  bf16 chains of <=4 steps cost ~4e-4 final
  rel err vs the 2e-2 tolerance.)

Exp and Derivative_Erf live in different ACT table sets (~2.7us/switch), so
ACT batches one full iteration: [Exp: 12x tau][DErf: 12x (seeds+dense)].

Per-partition partials land in hacc[128, 192]; two PE matmuls against the
4-block selector reduce over the 32 rows of each block; the tiny tail
(normalize, cumsum, |diff|, mean) runs on host in float64.
"""

import functools
import math

import numpy as np

N_CORES = 8
NUM_BINS = 64
B, C, H, W = 8, 3, 384, 384
HW = H * W
N_UNITS = 2 * C                 # (channel, pred/target) images per core
NREP = 4                        # replication factor
ROWS = 128 // NREP              # 32 image rows per replica block
IMG_FREE = HW // ROWS           # 4608
NQ = 2                          # halves per image
QF = IMG_FREE // NQ             # 2304
UNITS = N_UNITS * NQ            # 12 work units per core per iteration
NROUNDS = NUM_BINS // NREP      # 16 rounds per unit

DENOM = 2.0 * (1.0 / 64.0) ** 2 + 1e-7
SCALE = 1.0 / math.sqrt(DENOM)          # Derivative_Erf arg scale
SPACING = 1.0 / 63.0                    # bin-center spacing
TAU_SCALE = 2.0 * SPACING / DENOM       # tau = exp(TAU_SCALE * x)
DERF_SCALE = math.sqrt(math.pi) / 2.0   # Derivative_Erf = 2/sqrt(pi)*exp(-u^2)

SEED_ROUNDS = (0, 8)
# per-unit round split: even units run 8 DVE chain rounds, odd units 7,
# balancing ACT (~1.8us/pass) against DVE (~2.4us/round)
DENSE_N6 = (5, 6, 7, 13, 14, 15)
CHAIN_N6 = (1, 2, 3, 4, 9, 10, 11, 12)
DENSE_N7 = (4, 5, 6, 7, 13, 14, 15)
CHAIN_N7 = (1, 2, 3, 9, 10, 11, 12)

HACC_COLS = UNITS * NROUNDS     # 192

# consts column layout
CBIAS = {r: i for i, r in enumerate(SEED_ROUNDS + DENSE_N7)}  # DErf biases
CC = {r: 8 + r for r in range(1, 13)}       # C_{j-1->j} cols 9..20
CSEL = 21                                   # selector cols 21..24
NCONST = 28


def _n6(g):
    return (g % UNITS) % 2 == 0


def _unit_dense(g):
    return DENSE_N6 if _n6(g) else DENSE_N7


def _unit_act_kinds(g):
    return ("s0", "s8") + tuple(f"d{r}" for r in _unit_dense(g))


def _unit_dve_ops(g):
    return tuple(f"st{r}" for r in (CHAIN_N6 if _n6(g) else CHAIN_N7))


def _act_rounds(g):
    return ("tau",) + _unit_act_kinds(g)


ACT_LAST = "d15"
DVE_LAST = "st12"               # last chain op for both unit flavors
GROUP = 12                      # units per ACT table-set batch (one iteration)


def _build_schedules(R):
    act_prog, dve_prog = [], []
    for it in range(R):
        base = UNITS * it
        for grp in range(UNITS // GROUP):
            gs = [base + GROUP * grp + j for j in range(GROUP)]
            for g in gs:
                act_prog.append(("tau", g))
            for g in gs:
                act_prog += [(k, g) for k in _unit_act_kinds(g)]
        for u in range(UNITS):
            g = base + u
            dve_prog += [(k, g) for k in _unit_dve_ops(g)]
    act_index = {key: idx for idx, key in enumerate(act_prog)}
    dve_index = {key: idx for idx, key in enumerate(dve_prog)}
    return act_prog, dve_prog, act_index, dve_index


def _build_program(R=1):
    import concourse.bass as bass
    import concourse.mybir as mybir
    from contextlib import ExitStack

    act_prog, dve_prog, act_index, dve_index = _build_schedules(R)
    act_per_it = len(act_prog) // R
    dve_per_it = len(dve_prog) // R

    nc = bass.Bass()
    xs = [
        nc.dram_tensor(
            f"x{u}", [128, IMG_FREE], mybir.dt.float32, kind="ExternalInput"
        )
        for u in range(N_UNITS)
    ]
    cst = nc.dram_tensor(
        "consts", [128, NCONST], mybir.dt.float32, kind="ExternalInput"
    )
    hist_out = nc.dram_tensor(
        "hist", [128, 8], mybir.dt.float32, kind="ExternalOutput"
    )

    mult = mybir.AluOpType.mult
    bf16 = mybir.dt.bfloat16

    with ExitStack() as stack:
        def sb(name, shape, dt=mybir.dt.float32):
            return stack.enter_context(nc.sbuf_tensor(name, shape, dt))

        xts = [sb(f"xt{i}", [128, QF]) for i in range(12)]
        taus = [sb(f"tau{i}", [128, QF], bf16) for i in range(12)]
        sas = [sb(f"sa{i}", [128, QF], bf16) for i in range(2)]
        sbs = [sb(f"sb{i}", [128, QF], bf16) for i in range(2)]
        pps = [sb(f"pp{i}", [128, QF], bf16) for i in range(2)]
        wscr = sb("wscr", [128, QF], bf16)
        cstt = sb("cstt", [128, NCONST])
        hacc = sb("hacc", [128, HACC_COLS])
        ho = sb("ho", [128, 8])
        ph0 = stack.enter_context(nc.psum_tensor("ph0", [128, 4], mybir.dt.float32))
        ph1 = stack.enter_context(nc.psum_tensor("ph1", [64, 4], mybir.dt.float32))
        sem_c = stack.enter_context(nc.semaphore("sem_c"))
        xsems = [stack.enter_context(nc.semaphore(f"sem_x{i}")) for i in range(12)]
        act_sem = stack.enter_context(nc.semaphore("act_sem"))
        dve_sem = stack.enter_context(nc.semaphore("dve_sem"))
        pe_sem = stack.enter_context(nc.semaphore("pe_sem"))
        cp_sem = stack.enter_context(nc.semaphore("cp_sem"))
        block = stack.enter_context(nc.Block())

        def haccol(g, rnd):
            c = NROUNDS * (g % UNITS) + rnd
            return hacc[:, c : c + 1]

        @block.sync
        def _(sync):
            sync.dma_start(out=cstt[:], in_=cst[:]).then_inc(sem_c, 16)
            for it in range(R):
                for u in range(UNITS):
                    g = UNITS * it + u
                    slot = g % 12
                    i, q = (g % UNITS) // NQ, g % NQ
                    if g >= 12:
                        sync.wait_ge(act_sem, act_index[(ACT_LAST, g - 12)] + 1)
                    sync.dma_start(
                        out=xts[slot][:], in_=xs[i][:, QF * q : QF * (q + 1)]
                    ).then_inc(xsems[slot], 16)
                sync.wait_ge(cp_sem, 2 * (it + 1))
                sync.dma_start(out=hist_out[:], in_=ho[:]).then_inc(sem_c, 16)

        @block.scalar
        def _(scalar):
            # dummy activation: pulls the exp table load forward
            scalar.activation(
                wscr[0:128, 0:1], wscr[0:128, 1:2],
                mybir.ActivationFunctionType.Exp,
                bias=0.0, scale=1.0,
            )
            scalar.wait_ge(sem_c, 16)
            for kind, g in act_prog:
                slot, b = g % 12, g % 2
                tslot = g % 12
                it = g // UNITS
                scalar.wait_ge(xsems[slot], 16 * (g // 12 + 1))
                if kind == "tau":
                    if g >= 12:
                        scalar.wait_ge(dve_sem, dve_index[(DVE_LAST, g - 12)] + 1)
                    ins = scalar.activation(
                        taus[tslot][:], xts[slot][:],
                        mybir.ActivationFunctionType.Exp,
                        bias=0.0, scale=float(TAU_SCALE),
                    )
                elif kind in ("s0", "s8"):
                    rnd = int(kind[1:])
                    if kind == "s0" and g % UNITS == 0 and it > 0:
                        scalar.wait_ge(pe_sem, it)
                    if g >= 2:
                        scalar.wait_ge(dve_sem, dve_index[(DVE_LAST, g - 2)] + 1)
                    dst = sas[b] if kind == "s0" else sbs[b]
                    cb = CBIAS[rnd]
                    ins = scalar.activation(
                        dst[:], xts[slot][:],
                        mybir.ActivationFunctionType.Derivative_Erf,
                        bias=cstt[:, cb : cb + 1], scale=float(SCALE),
                        accum_out=haccol(g, rnd),
                    )
                else:  # dense d6/d7/d15
                    rnd = int(kind[1:])
                    cb = CBIAS[rnd]
                    ins = scalar.activation(
                        wscr[:], xts[slot][:],
                        mybir.ActivationFunctionType.Derivative_Erf,
                        bias=cstt[:, cb : cb + 1], scale=float(SCALE),
                        accum_out=haccol(g, rnd),
                    )
                ins.then_inc(act_sem, 1)

        @block.vector
        def _(vector):
            for kind, g in dve_prog:
                tslot, b = g % 12, g % 2
                rnd = int(kind[2:])
                chain = "A" if rnd < 8 else "B"
                step = rnd - (0 if chain == "A" else 8)   # 1..3 / 1..4
                base = sas[b] if chain == "A" else sbs[b]
                src = base if step % 2 == 1 else pps[b]
                dst = pps[b] if step % 2 == 1 else base
                if rnd == 1:
                    vector.wait_ge(act_sem, act_index[("s0", g)] + 1)
                elif rnd == 9:
                    vector.wait_ge(act_sem, act_index[("s8", g)] + 1)
                cc = CC[rnd]
                # w' = (w * C_r) * tau  with fused fp32 accum
                ins = vector.scalar_tensor_tensor(
                    dst[:], src[:], cstt[:, cc : cc + 1], taus[tslot][:],
                    mult, mult,
                    accum_out=haccol(g, rnd),
                )
                ins.then_inc(dve_sem, 1)
                if kind == DVE_LAST and g % UNITS == UNITS - 1:
                    it = g // UNITS
                    vector.wait_ge(pe_sem, it + 1)
                    vector.wait_ge(sem_c, 16 * (it + 1))
                    vector.tensor_copy(ho[:, 0:4], ph0[:, :]).then_inc(cp_sem, 1)
                    vector.tensor_copy(ho[0:64, 4:8], ph1[:, :]).then_inc(
                        cp_sem, 1
                    )

        @block.tensor
        def _(tensor):
            for it in range(R):
                tensor.wait_ge(act_sem, act_per_it * (it + 1))
                tensor.wait_ge(dve_sem, dve_per_it * (it + 1))
                tensor.matmul(
                    ph0[0:128, 0:4], hacc[:, 0:128], cstt[:, CSEL : CSEL + 4],
                    start=True, stop=True,
                )
                tensor.matmul(
                    ph1[0:64, 0:4], hacc[:, 128:192], cstt[:, CSEL : CSEL + 4],
                    start=True, stop=True,
                ).then_inc(pe_sem, 1)

    return nc


def _make_consts():
    centers = np.linspace(0.0, 1.0, NUM_BINS).astype(np.float64)
    p = np.arange(128)
    k = p // ROWS
    cst = np.zeros((128, NCONST), dtype=np.float64)
    for r, ci in CBIAS.items():
        cst[:, ci] = -centers[NROUNDS * k + r] * SCALE
    for r, ci in CC.items():
        j = NROUNDS * k + r
        cst[:, ci] = np.exp(-SPACING * (centers[j - 1] + centers[j]) / DENOM)
    for kk in range(NREP):
        cst[k == kk, CSEL + kk] = 1.0
    return cst.astype(np.float32)


@functools.lru_cache(maxsize=1)
def _get_runner():
    """Compile the SPMD program once; return a callable list[in_map] -> list[out_map]."""
    import jax
    from jax.experimental.shard_map import shard_map
    from jax.sharding import Mesh, PartitionSpec

    from concourse import mybir
    from concourse.bass2jax import (
        _bass_exec_p,
        install_neuronx_cc_hook,
        partition_id_tensor,
    )

    nc = _build_program()
    install_neuronx_cc_hook()

    partition_name = (
        nc.partition_id_tensor.name if nc.partition_id_tensor else None
    )
    in_names, out_names, out_avals, zero_outs = [], [], [], []
    for alloc in nc.m.functions[0].allocations:
        if not isinstance(alloc, mybir.MemoryLocationSet):
            continue
        name = alloc.memorylocations[0].name
        if alloc.kind == "ExternalInput":
            if name != partition_name:
                in_names.append(name)
        elif alloc.kind == "ExternalOutput":
            out_names.append(name)
            shape = tuple(alloc.tensor_shape)
            dtype = mybir.dt.np(alloc.dtype)
            out_avals.append(jax.core.ShapedArray(shape, dtype))
            zero_outs.append(np.zeros(shape, dtype))
    n_params = len(in_names)
    n_outs = len(out_avals)
    all_in_names = list(in_names) + list(out_names)
    if partition_name is not None:
        all_in_names.append(partition_name)
    donate = tuple(range(n_params, n_params + n_outs))

    def _body(*args):
        operands = list(args)
        if partition_name is not None:
            operands.append(partition_id_tensor())
        outs = _bass_exec_p.bind(
            *operands,
            out_avals=tuple(out_avals),
            in_names=tuple(all_in_names),
            out_names=tuple(out_names),
            lowering_input_output_aliases=(),
            sim_require_finite=True,
            sim_require_nnan=True,
            nc=nc,
        )
        return tuple(outs)

    devices = jax.devices()[:N_CORES]
    mesh = Mesh(np.asarray(devices), ("core",))
    sharded = jax.jit(
        shard_map(
            _body,
            mesh=mesh,
            in_specs=(PartitionSpec("core"),) * (n_params + n_outs),
            out_specs=(PartitionSpec("core"),) * n_outs,
            check_rep=False,
        ),
        donate_argnums=donate,
        keep_unused=True,
    )

    class Runner:
        def __init__(self):
            self.sharded = sharded
            self.in_names = in_names
            self.out_names = out_names
            self.out_avals = out_avals
            self.zero_outs = zero_outs

        def concat_inputs(self, in_maps):
            return [
                np.concatenate([np.asarray(m[name]) for m in in_maps], axis=0)
                for name in in_names
            ]

        def fresh_zeros(self):
            return [
                np.zeros((N_CORES * z.shape[0], *z.shape[1:]), z.dtype)
                for z in zero_outs
            ]

        def split_outputs(self, out_arrs):
            return [
                {
                    name: np.asarray(out_arrs[i]).reshape(
                        N_CORES, *out_avals[i].shape
                    )[c]
                    for i, name in enumerate(out_names)
                }
                for c in range(N_CORES)
            ]

        def __call__(self, in_maps):
            out_arrs = self.sharded(*self.concat_inputs(in_maps), *self.fresh_zeros())
            return self.split_outputs(out_arrs)

    return Runner()


def _shard_inputs(pred, target):
    cst = _make_consts()
    maps = []
    for b in range(B):
        m = {"consts": cst}
        for c in range(C):
            for t, src in enumerate((pred, target)):
                u = 2 * c + t
                img = np.ascontiguousarray(src[b, c], dtype=np.float32).reshape(
                    ROWS, IMG_FREE
                )
                m[f"x{u}"] = np.tile(img, (NREP, 1))
        maps.append(m)
    return maps


def _unpack_hist(ho):
    """ho [128, 8] -> hist [N_UNITS, NUM_BINS] (float64)."""
    ho = ho.astype(np.float64)
    hist = np.zeros((N_UNITS, NUM_BINS), dtype=np.float64)
    for u in range(UNITS):
        i = u // NQ
        for rnd in range(NROUNDS):
            c = NROUNDS * u + rnd
            vals = ho[c, 0:4] if c < 128 else ho[c - 128, 4:8]
            for kk in range(NREP):
                hist[i, NROUNDS * kk + rnd] += vals[kk]
    return hist


def _finish_on_host(results):
    total = 0.0
    for b in range(B):
        hist = _unpack_hist(results[b]["hist"]) * DERF_SCALE
        for c in range(C):
            pcs = hist[2 * c]
            tcs = hist[2 * c + 1]
            pn = pcs / (pcs.sum() + 1e-7)
            tn = tcs / (tcs.sum() + 1e-7)
            total += np.abs(np.cumsum(pn) - np.cumsum(tn)).sum()
    return np.float32(total / (B * C * NUM_BINS))


def kernel(pred, target):
    pred = np.asarray(pred, dtype=np.float32)
    target = np.asarray(target, dtype=np.float32)
    assert pred.shape == (B, C, H, W) and target.shape == (B, C, H, W)
    run = _get_runner()
    results = run(_shard_inputs(pred, target))
    return np.asarray(_finish_on_host(results), dtype=np.float32)
